# revision 8
# baseline (speedup 1.0000x reference)
"""Trainium2 Bass kernel for 4D convolution (B=1, IC=16, OC=32, K=3^4, D=32^4,
stride 1, pad 1, dil 1) sharded over 8 NeuronCores along the first spatial dim.

Strategy
--------
Host side builds, per core, a 128-partition "shift stack": 8 copies of the
padded input slab, copy c = (d3, d4) in {0,1,2}^2 minus {(2,2)}, pre-shifted by
(d3, d4) along the last two spatial dims, laid out as partitions
p = 16*c + ic.  A single matmul with this stack as the moving operand then
contracts K = 128 = (ic, k3, k4) for one (k1, k2) kernel-offset pair — the
(k1, k2, x2, x1) shifts are plain access-pattern offsets.  Per output block
[32 oc x 512 positions] that leaves 9 K=128 matmuls + 9 K=16 "leftover"
matmuls for (k3,k4)=(2,2), which are packed 12-16x concurrent on the PE via
tile_position row/col groups.  Inputs are cast to fp16 (matmul runs at full
rate vs 4x slower for fp32; products accumulate in fp32 PSUM, so only the
input rounding (~2^-11) enters the error).

Per core: 4 output x1 slices, x2 processed in 8 windows of 4 with a ring of
stack slice-columns double-buffered against the matmuls; PSUM holds
4 col-groups x 8 banks = 32 blocks in flight; bank drains (with fused bias
add) are split between VectorE and ScalarE; one 2 MB output DMA per window.
"""

import numpy as np

import concourse.bass as bass
import concourse.mybir as mybir
from concourse import bacc, tile
from concourse.bass_utils import run_bass_kernel_spmd

FP16 = mybir.dt.float16
FP32 = mybir.dt.float32

N_CORES = 8
IC, OC = 16, 32
D = 32
X1_PER_CORE = D // N_CORES          # 4
N_SLABS = X1_PER_CORE + 2           # 6 padded-x1 slabs per core
NWIN = D // 4                       # 8 x2 windows of 4
# copy index c = 3*d3 + d4 for (d3, d4) != (2, 2); leftover row slots use the
# even copies c = 0, 2, 4, 6 (partition bases 0, 32, 64, 96).
LEFT_SLOTS = {0: (0, 0), 1: (0, 2), 2: (1, 1), 3: (2, 0)}  # r -> (d3, d4)

_CACHE = {}


def _build_program():
    nc = bacc.Bacc("TRN2", target_bir_lowering=False, debug=False,
                   enable_asserts=False, num_devices=N_CORES)
    x_stack = nc.dram_tensor("x_stack", [128, N_SLABS, 34, 34, 34], FP16,
                             kind="ExternalInput")
    w_big = nc.dram_tensor("w_big", [128, 9, 32], FP16, kind="ExternalInput")
    w_left = nc.dram_tensor("w_left", [16, 9, 32], FP16, kind="ExternalInput")
    bias_in = nc.dram_tensor("bias_in", [128, 1], FP32, kind="ExternalInput")
    out = nc.dram_tensor("out", [OC, X1_PER_CORE, D, D, D], FP32,
                         kind="ExternalOutput")

    with tile.TileContext(nc) as tc:
        with tc.tile_pool(name="wpool", bufs=1) as wpool, \
             tc.tile_pool(name="spool", bufs=8) as spool, \
             tc.tile_pool(name="opool", bufs=2) as opool, \
             tc.tile_pool(name="ppool", bufs=1, space="PSUM") as ppool:

            wb = wpool.tile([128, 9, 32], FP16)
            nc.sync.dma_start(wb[:], w_big[:])
            wl = wpool.tile([128, 9, 32], FP16)
            for r in range(4):
                nc.sync.dma_start(wl[32 * r:32 * r + 16, :, :], w_left[:])
            bs = wpool.tile([128, 1], FP32)
            nc.sync.dma_start(bs[:], bias_in[:])

            psum = [ppool.tile([128, 512], FP32, name=f"ps{b}", tag=f"ps{b}")
                    for b in range(8)]

            stiles = {}

            def load_col(x2s):
                t = spool.tile([128, N_SLABS, 34, 34], FP16, tag="scol",
                               name=f"sc{x2s}")
                nc.sync.dma_start(t[:], x_stack[:, :, x2s, :, :])
                stiles[x2s] = t

            for s in range(6):
                load_col(s)

            for w in range(NWIN):
                # prefetch next window's new slice-columns
                for s in range(4 * w + 6, min(4 * w + 10, 34)):
                    load_col(s)

                for k2 in range(3):
                    for k1 in range(3):
                        k12 = 3 * k1 + k2
                        start = (k2 == 0 and k1 == 0)
                        for b in range(8):
                            x1o, h = divmod(b, 2)   # h = x3-half
                            for j in range(4):
                                st = stiles[4 * w + j + k2]
                                rhs = st[:, x1o + k1, 16 * h:16 * h + 16, 0:32]
                                nc.tensor.matmul(
                                    psum[b][32 * j:32 * j + 32, :],
                                    wb[:, k12, :], rhs,
                                    start=start, stop=False,
                                    skip_group_check=True,
                                    tile_position=(0, 32 * j))
                    # leftover (k3,k4)=(2,2) for this k2, spread over row slots
                    for k1 in range(3):
                        k12 = 3 * k1 + k2
                        r = k12 % 4
                        d3, d4 = LEFT_SLOTS[r]
                        o3, o4 = 2 - d3, 2 - d4
                        stop = (k2 == 2 and k1 == 2)
                        for b in range(8):
                            x1o, h = divmod(b, 2)
                            for j in range(4):
                                st = stiles[4 * w + j + k2]
                                rhs = st[32 * r:32 * r + 16, x1o + k1,
                                         16 * h + o3:16 * h + o3 + 16,
                                         o4:o4 + 32]
                                nc.tensor.matmul(
                                    psum[b][32 * j:32 * j + 32, :],
                                    wl[32 * r:32 * r + 16, k12, :], rhs,
                                    start=False, stop=stop,
                                    skip_group_check=True,
                                    tile_position=(32 * r, 32 * j))

                # drain PSUM banks (bias fused), split across DVE and ACT
                ot = opool.tile([128, 4, 1024], FP32, tag="ot", name=f"ot{w}")
                for b in range(8):
                    x1o, h = divmod(b, 2)
                    dst = ot[:, x1o, 512 * h:512 * h + 512]
                    if b % 2 == 0:
                        nc.vector.tensor_scalar_add(
                            dst, psum[b][:, :], bs[:, 0:1])
                    else:
                        nc.scalar.activation(
                            dst, psum[b][:, :],
                            mybir.ActivationFunctionType.Identity,
                            bias=bs[:, 0:1])

                for j in range(4):
                    dstj = out[:, :, 4 * w + j, :, :].rearrange(
                        "oc x1 x3 x4 -> oc x1 (x3 x4)")
                    nc.sync.dma_start(dstj, ot[32 * j:32 * j + 32, :, :])

    nc.compile()
    return nc


def _prep_inputs(inputs, weight, bias):
    """Host-side shard + shift-stack construction."""
    x = np.asarray(inputs)[0]                       # [16, 32, 32, 32, 32]
    xp = np.pad(x, ((0, 0), (1, 1), (1, 1), (1, 3), (1, 3)))  # [16,34,34,36,36]
    xp16 = xp.astype(np.float16)

    w6 = np.asarray(weight).reshape(OC, IC, 3, 3, 3, 3)
    w_big = np.empty((128, 9, 32), np.float16)
    for c in range(8):
        d3, d4 = divmod(c, 3)
        # w_big[16c+ic, 3*k1+k2, oc] = w6[oc, ic, k1, k2, d3, d4]
        w_big[16 * c:16 * c + 16] = (
            w6[:, :, :, :, d3, d4].reshape(OC, IC, 9)
            .transpose(1, 2, 0).astype(np.float16))
    w_left = (w6[:, :, :, :, 2, 2].reshape(OC, IC, 9)
              .transpose(1, 2, 0).astype(np.float16).copy())

    b = np.asarray(bias).reshape(OC).astype(np.float32)
    bias_rep = np.tile(b, 4).reshape(128, 1).copy()

    in_maps = []
    for core in range(N_CORES):
        xc = xp16[:, 4 * core:4 * core + N_SLABS]   # [16, 6, 34, 36, 36]
        st = np.empty((128, N_SLABS, 34, 34, 34), np.float16)
        for c in range(8):
            d3, d4 = divmod(c, 3)
            st[16 * c:16 * c + 16] = xc[:, :, :, d3:d3 + 34, d4:d4 + 34]
        in_maps.append({"x_stack": st, "w_big": w_big, "w_left": w_left,
                        "bias_in": bias_rep})
    return in_maps


def run_on_hw(inputs, weight, bias, trace=False):
    if "nc" not in _CACHE:
        _CACHE["nc"] = _build_program()
    nc = _CACHE["nc"]
    in_maps = _prep_inputs(inputs, weight, bias)
    res = run_bass_kernel_spmd(nc, in_maps, list(range(N_CORES)), trace=trace)
    parts = [res.results[c]["out"] for c in range(N_CORES)]  # [32,4,32,32,32]
    full = np.concatenate(parts, axis=1)[None]               # [1,32,32,32,32,32]
    return np.ascontiguousarray(full, dtype=np.float32), res


def kernel(inputs, weight, bias):
    out, _ = run_on_hw(inputs, weight, bias, trace=False)
    return out
